# revision 5
# baseline (speedup 1.0000x reference)
"""Trainium2 Bass kernel for ChannelMixingKAN.

Model: LN over (T,C) per batch -> KANLinear(C=128 -> H=256) -> KANLinear(H=256 -> C=128)
with cubic B-spline bases (grid 5, order 3, range [-1,1]) -> residual.

Strategy:
  - Data-parallel over batch: 64 batches -> 8 cores x 8 batches.
  - Everything stays in the native (C, T) layout: U[b] is (128, 512) with C on
    partitions, so LN, both KAN layers and the residual need no transposes.
  - The 8 cubic B-spline basis functions of s = (x + 2.2) / 0.4 are generated
    from 16 "tent-cube" features per input feature:
        basis_j(s) = relu(2 - |s - (j+2)|)^3 / 6  -  (2/3) relu(1 - |s - (j+2)|)^3
    which is exact (verified vs the Cox-de Boor recursion).  The cube halves are
    produced as  F_j = msq*m (= -a^3/6),  G_j = nsq*n (= -(2/3)b^3)  with
        v = |s - (j+2)|;  m = min(v-2, 0);  n = min(v-1, 0)
        msq = (sqrt(1/6) m)^2  (ScalarE Square);  nsq = (sqrt(2/3) n)^2
    and the +-1 signs are folded into the weights.
  - Matmuls run in fp16 (full PE rate); contraction = 17 chunks of 128 per
    layer half: [silu(x), F_0, G_0, ..., F_7, G_7] x per-chunk weight slices.
  - PSUM accumulates in fp32; LN statistics and the residual stay fp32.
"""

import numpy as np

B, C, T, H = 64, 128, 512, 256
N_CORES = 8
BPC = B // N_CORES            # batches per core
LN_EPS = 1e-5
NCHUNK = 17                   # silu + 16 tent-cube features
SQ_M = 0.4082482904638631     # sqrt(1/6)
SQ_N = 0.816496580927726      # sqrt(2/3)

_CACHE = {}


def _build(ln_affine: bool):
    import concourse.bacc as bacc
    import concourse.tile as tile
    from concourse import mybir

    F16 = mybir.dt.float16
    U16 = mybir.dt.uint16
    F32 = mybir.dt.float32
    Op = mybir.AluOpType
    Act = mybir.ActivationFunctionType

    nc = bacc.Bacc(None, target_bir_lowering=False)

    u_ext = nc.declare_dram_parameter("u", [BPC, C, T], F32, isOutput=False)
    w1_ext = nc.declare_dram_parameter("w1", [NCHUNK, C, H], F16, isOutput=False)
    w2_ext = nc.declare_dram_parameter("w2", [2, NCHUNK, 128, C], F16, isOutput=False)
    if ln_affine:
        lnw_ext = nc.declare_dram_parameter("lnw", [C, T], F32, isOutput=False)
        lnb_ext = nc.declare_dram_parameter("lnb", [C, T], F32, isOutput=False)
    out_ext = nc.declare_dram_parameter("out", [BPC, C, T], F32, isOutput=True)

    from contextlib import ExitStack

    with tile.TileContext(nc) as tc, ExitStack() as ctx:
        singles = ctx.enter_context(tc.tile_pool(name="singles", bufs=1))
        u_pool = ctx.enter_context(tc.tile_pool(name="u", bufs=BPC))
        st_pool = ctx.enter_context(tc.tile_pool(name="st", bufs=2))
        f1_pool = ctx.enter_context(tc.tile_pool(name="f1", bufs=2))
        f2_pool = ctx.enter_context(tc.tile_pool(name="f2", bufs=4))
        s_pool = ctx.enter_context(tc.tile_pool(name="s", bufs=6))
        scr_pool = ctx.enter_context(tc.tile_pool(name="scr", bufs=2))
        o_pool = ctx.enter_context(tc.tile_pool(name="o", bufs=2))
        psum = ctx.enter_context(tc.tile_pool(name="psum", bufs=4, space="PSUM"))
        psum2 = ctx.enter_context(tc.tile_pool(name="psum2", bufs=2, space="PSUM"))
        psum_s = ctx.enter_context(tc.tile_pool(name="psum_s", bufs=2, space="PSUM"))

        # ---- constants & weights ----
        w1_sb = singles.tile([C, NCHUNK, H], F16)
        nc.sync.dma_start(out=w1_sb[:], in_=w1_ext.rearrange("k p m -> p k m"))
        w2_sb = singles.tile([C, 2, NCHUNK, C], F16)
        nc.sync.dma_start(out=w2_sb[:], in_=w2_ext.rearrange("g k p m -> p g k m"))
        ones_sb = singles.tile([128, 128], F32)
        nc.vector.memset(ones_sb[:], 1.0 / 128.0)
        eps_sb = singles.tile([128, 1], F32)
        nc.vector.memset(eps_sb[:], LN_EPS)
        if ln_affine:
            lnw_sb = singles.tile([C, T], F32)
            nc.sync.dma_start(out=lnw_sb[:], in_=lnw_ext[:])
            lnb_sb = singles.tile([C, T], F32)
            nc.sync.dma_start(out=lnb_sb[:], in_=lnb_ext[:])

        # per-batch LN coefficients: [inv, a_s, b_s, nb]
        coef = singles.tile([128, BPC, 4], F32)

        u_tiles = []
        for b in range(BPC):
            u_t = u_pool.tile([C, T], F32, tag="u")
            nc.sync.dma_start(out=u_t[:], in_=u_ext[b])
            u_tiles.append(u_t)

            # ---- LN stats for batch b ----
            st = st_pool.tile([128, 6], F32, tag="bnst")
            nc.vector.bn_stats(out=st[:], in_=u_t[:])
            mv = st_pool.tile([128, 2], F32, tag="bnmv")
            nc.vector.bn_aggr(out=mv[:], in_=st[:])
            m2 = st_pool.tile([128, 1], F32, tag="m2")
            nc.vector.tensor_tensor(out=m2[:], in0=mv[:, 0:1], in1=mv[:, 0:1], op=Op.mult)
            # mv[:,1] := var_p + mean_p^2  (per-partition E[x^2])
            nc.vector.tensor_tensor(out=mv[:, 1:2], in0=mv[:, 1:2], in1=m2[:], op=Op.add)
            ps = psum_s.tile([128, 2], F32, tag="ps_st")
            nc.tensor.matmul(ps[:], lhsT=ones_sb[:], rhs=mv[:], start=True, stop=True)
            # ps[:,0] = mean_tot ; ps[:,1] = E[x^2]_tot  (broadcast to all partitions)
            stot = st_pool.tile([128, 2], F32, tag="stot")
            nc.vector.tensor_copy(out=stot[:], in_=ps[:])
            var = st_pool.tile([128, 1], F32, tag="var")
            nc.vector.tensor_tensor(out=var[:], in0=stot[:, 0:1], in1=stot[:, 0:1], op=Op.mult)
            nc.vector.tensor_tensor(out=var[:], in0=stot[:, 1:2], in1=var[:], op=Op.subtract)
            sd = st_pool.tile([128, 1], F32, tag="sd")
            nc.scalar.activation(out=sd[:], in_=var[:], func=Act.Sqrt, bias=eps_sb[:])
            inv = coef[:, b, 0:1]
            nc.vector.reciprocal(out=inv, in_=sd[:])
            mi = st_pool.tile([128, 1], F32, tag="mi")
            nc.vector.tensor_tensor(out=mi[:], in0=stot[:, 0:1], in1=inv, op=Op.mult)
            # a_s = 2.5*inv ; b_s = -2.5*mu*inv + 5.5 ; nb = -mu*inv
            nc.vector.tensor_scalar(out=coef[:, b, 1:2], in0=inv, scalar1=2.5,
                                    scalar2=None, op0=Op.mult)
            nc.vector.tensor_scalar(out=coef[:, b, 2:3], in0=mi[:], scalar1=-2.5,
                                    scalar2=5.5, op0=Op.mult, op1=Op.add)
            nc.vector.tensor_scalar(out=coef[:, b, 3:4], in0=mi[:], scalar1=-1.0,
                                    scalar2=None, op0=Op.mult)

        def emit_features(s_ap, feats, scr_tag):
            """feats[:, 1+2j, :] = -a^3/6, feats[:, 2+2j, :] = -(2/3)b^3."""
            for j in range(8):
                d = scr_pool.tile([128, T], F16, tag=f"{scr_tag}d")
                nc.vector.tensor_scalar(out=d[:], in0=s_ap, scalar1=float(j + 2),
                                        scalar2=None, op0=Op.subtract)
                v = scr_pool.tile([128, T], F16, tag=f"{scr_tag}v")
                nc.vector.tensor_scalar(out=v.bitcast(U16)[:], in0=d.bitcast(U16)[:],
                                        scalar1=0x7FFF, scalar2=None, op0=Op.bitwise_and)
                m = scr_pool.tile([128, T], F16, tag=f"{scr_tag}m")
                nc.vector.tensor_scalar(out=m[:], in0=v[:], scalar1=2.0,
                                        scalar2=0.0, op0=Op.subtract, op1=Op.min)
                n = scr_pool.tile([128, T], F16, tag=f"{scr_tag}n")
                nc.vector.tensor_scalar(out=n[:], in0=m[:], scalar1=1.0,
                                        scalar2=0.0, op0=Op.add, op1=Op.min)
                msq = scr_pool.tile([128, T], F16, tag=f"{scr_tag}msq")
                nc.scalar.activation(out=msq[:], in_=m[:], func=Act.Square, scale=SQ_M)
                nsq = scr_pool.tile([128, T], F16, tag=f"{scr_tag}nsq")
                nc.scalar.activation(out=nsq[:], in_=n[:], func=Act.Square, scale=SQ_N)
                nc.vector.tensor_tensor(out=feats[:, 1 + 2 * j, :], in0=msq[:],
                                        in1=m[:], op=Op.mult)
                nc.vector.tensor_tensor(out=feats[:, 2 + 2 * j, :], in0=nsq[:],
                                        in1=n[:], op=Op.mult)

        for b in range(BPC):
            u_t = u_tiles[b]
            inv = coef[:, b, 0:1]
            a_s = coef[:, b, 1:2]
            b_s = coef[:, b, 2:3]
            nb = coef[:, b, 3:4]

            feats1 = f1_pool.tile([128, NCHUNK, T], F16, tag="feats1")
            if not ln_affine:
                # s1 = 2.5*(x-mu)*inv + 5.5 ; silu1 = silu((x-mu)*inv)
                s1 = s_pool.tile([128, T], F16, tag="s1")
                nc.vector.tensor_scalar(out=s1[:], in0=u_t[:], scalar1=a_s,
                                        scalar2=b_s, op0=Op.mult, op1=Op.add)
                nc.scalar.activation(out=feats1[:, 0, :], in_=u_t[:], func=Act.Silu,
                                     bias=nb, scale=inv)
            else:
                z = s_pool.tile([128, T], F32, tag="z")
                nc.vector.tensor_scalar(out=z[:], in0=u_t[:], scalar1=inv,
                                        scalar2=nb, op0=Op.mult, op1=Op.add)
                nc.vector.tensor_tensor(out=z[:], in0=z[:], in1=lnw_sb[:], op=Op.mult)
                nc.vector.tensor_tensor(out=z[:], in0=z[:], in1=lnb_sb[:], op=Op.add)
                s1 = s_pool.tile([128, T], F16, tag="s1")
                nc.vector.tensor_scalar(out=s1[:], in0=z[:], scalar1=2.5,
                                        scalar2=5.5, op0=Op.mult, op1=Op.add)
                nc.scalar.activation(out=feats1[:, 0, :], in_=z[:], func=Act.Silu)

            emit_features(s1[:], feats1, "l1")

            # ---- layer 1 matmuls ----
            s2 = []
            feats2 = []
            for h in range(2):
                ps1 = psum.tile([128, T], F32, tag="ps1")
                for k in range(NCHUNK):
                    nc.tensor.matmul(ps1[:], lhsT=w1_sb[:, k, h * 128:(h + 1) * 128],
                                     rhs=feats1[:, k, :],
                                     start=(k == 0), stop=(k == NCHUNK - 1))
                f2 = f2_pool.tile([128, NCHUNK, T], F16, tag="feats2")
                nc.scalar.activation(out=f2[:, 0, :], in_=ps1[:], func=Act.Silu)
                s2h = s_pool.tile([128, T], F16, tag="s2")
                nc.vector.tensor_scalar(out=s2h[:], in0=ps1[:], scalar1=2.5,
                                        scalar2=5.5, op0=Op.mult, op1=Op.add)
                s2.append(s2h)
                feats2.append(f2)

            for h in range(2):
                emit_features(s2[h][:], feats2[h], "l2")

            # ---- layer 2 matmuls ----
            ps2 = psum2.tile([128, T], F32, tag="ps2")
            for h in range(2):
                for k in range(NCHUNK):
                    nc.tensor.matmul(ps2[:], lhsT=w2_sb[:, h, k, :],
                                     rhs=feats2[h][:, k, :],
                                     start=(h == 0 and k == 0),
                                     stop=(h == 1 and k == NCHUNK - 1))

            # ---- residual + store ----
            o_t = o_pool.tile([128, T], F32, tag="o")
            nc.vector.tensor_tensor(out=o_t[:], in0=ps2[:], in1=u_t[:], op=Op.add)
            nc.sync.dma_start(out=out_ext[b], in_=o_t[:])

    nc.compile()
    return nc


def _get_nc(ln_affine: bool):
    key = ("nc", ln_affine)
    if key not in _CACHE:
        _CACHE[key] = _build(ln_affine)
    return _CACHE[key]


def _prep_weights(bw1, sw1, ss1, bw2, sw2, ss2):
    sw1s = sw1 * ss1[:, :, None]               # (H, C, 8)
    sw2s = sw2 * ss2[:, :, None]               # (C, H, 8)
    w1 = np.empty((NCHUNK, C, H), np.float16)
    w1[0] = bw1.T
    for j in range(8):
        w1[1 + 2 * j] = -sw1s[:, :, j].T
        w1[2 + 2 * j] = +sw1s[:, :, j].T
    w2 = np.empty((2, NCHUNK, 128, C), np.float16)
    for h in range(2):
        w2[h, 0] = bw2[:, h * 128:(h + 1) * 128].T
        for j in range(8):
            w2[h, 1 + 2 * j] = -sw2s[:, h * 128:(h + 1) * 128, j].T
            w2[h, 2 + 2 * j] = +sw2s[:, h * 128:(h + 1) * 128, j].T
    return w1, w2


def make_in_maps(U, ln_w, ln_b, bw1, sw1, ss1, bw2, sw2, ss2):
    U = np.ascontiguousarray(np.asarray(U, dtype=np.float32))
    ln_affine = not (np.all(ln_w == 1.0) and np.all(ln_b == 0.0))
    w1, w2 = _prep_weights(np.asarray(bw1, np.float32), np.asarray(sw1, np.float32),
                           np.asarray(ss1, np.float32), np.asarray(bw2, np.float32),
                           np.asarray(sw2, np.float32), np.asarray(ss2, np.float32))
    shards = U.reshape(N_CORES, BPC, C, T)
    in_maps = []
    for c in range(N_CORES):
        m = {"u": shards[c], "w1": w1, "w2": w2}
        if ln_affine:
            m["lnw"] = np.ascontiguousarray(np.asarray(ln_w, np.float32).T)
            m["lnb"] = np.ascontiguousarray(np.asarray(ln_b, np.float32).T)
        in_maps.append(m)
    return in_maps, ln_affine


def run_in_maps(in_maps, ln_affine):
    from concourse.bass_utils import run_bass_kernel_spmd
    nc = _get_nc(ln_affine)
    res = run_bass_kernel_spmd(nc, in_maps, core_ids=list(range(N_CORES)))
    return res


def kernel(U, ln_w, ln_b, bw1, sw1, ss1, bw2, sw2, ss2):
    in_maps, ln_affine = make_in_maps(U, ln_w, ln_b, bw1, sw1, ss1, bw2, sw2, ss2)
    res = run_in_maps(in_maps, ln_affine)
    out = np.concatenate([res.results[c]["out"] for c in range(N_CORES)], axis=0)
    return out.reshape(B, C, T).astype(np.float32)


# revision 8
# speedup vs baseline: 1.2053x; 1.2053x over previous
"""Trainium2 Bass kernel for ChannelMixingKAN.

Model: LN over (T,C) per batch -> KANLinear(C=128 -> H=256) -> KANLinear(H=256 -> C=128)
with cubic B-spline bases (grid 5, order 3, range [-1,1]) -> residual.

Strategy:
  - Data-parallel over batch: 64 batches -> 8 cores x 8 batches.
  - Everything stays in the native (C, T) layout: U[b] is (128, 512) with C on
    partitions, so LN, both KAN layers and the residual need no transposes.
  - The 8 cubic B-spline basis functions of s = (x + 2.2) / 0.4 are generated
    from 16 "tent-cube" features per input feature:
        basis_j(s) = relu(2 - |s - (j+2)|)^3 / 6  -  (2/3) relu(1 - |s - (j+2)|)^3
    (exact vs the Cox-de Boor recursion).  With m = min(|s-c_j|-2, 0) and
    n = min(m+1, 0):  F_j = (sqrt(1/6) m)^2 * m = -a^3/6,
    G_j = (sqrt(2/3) n)^2 * n = -(2/3) b^3, and signs fold into the weights.
  - Feature stages are fused across j (one (128, 8, 512) op per stage) and
    spread over three engines: per-j shifts on GPSIMD, |.| (uint16 AND) /
    clamp / cube-mults on DVE, squares on ScalarE.
  - Matmuls run in fp16 (full PE rate); contraction = 17 chunks of 128 per
    layer half: [silu, F_0..F_7, G_0..G_7].  PSUM accumulates fp32; LN stats
    and the residual stay fp32.
"""

import numpy as np

B, C, T, H = 64, 128, 512, 256
N_CORES = 8
BPC = B // N_CORES            # batches per core
LN_EPS = 1e-5
NCHUNK = 17                   # silu + 16 tent-cube features
SQ_M = 0.4082482904638631     # sqrt(1/6)
SQ_N = 0.816496580927726      # sqrt(2/3)

_CACHE = {}


def _build(ln_affine: bool, sim_safe: bool = False, trace_sim: bool = False,
           n_loop: int = 1):
    import concourse.bacc as bacc
    import concourse.tile as tile
    from concourse import mybir
    from contextlib import ExitStack, nullcontext

    F16 = mybir.dt.float16
    U16 = mybir.dt.uint16
    F32 = mybir.dt.float32
    Op = mybir.AluOpType
    Act = mybir.ActivationFunctionType
    ACT_SILU = Act.Sigmoid if sim_safe else Act.Silu

    nc = bacc.Bacc(None, target_bir_lowering=False)

    u_ext = nc.declare_dram_parameter("u", [BPC, C, T], F32, isOutput=False)
    w1_ext = nc.declare_dram_parameter("w1", [NCHUNK, C, H], F16, isOutput=False)
    w2_ext = nc.declare_dram_parameter("w2", [2, NCHUNK, 128, C], F16, isOutput=False)
    if ln_affine:
        lnw_ext = nc.declare_dram_parameter("lnw", [C, T], F32, isOutput=False)
        lnb_ext = nc.declare_dram_parameter("lnb", [C, T], F32, isOutput=False)
    out_ext = nc.declare_dram_parameter("out", [BPC, C, T], F32, isOutput=True)

    with tile.TileContext(nc, trace_sim=trace_sim) as tc, ExitStack() as ctx:
        singles = ctx.enter_context(tc.tile_pool(name="singles", bufs=1))
        u_pool = ctx.enter_context(tc.tile_pool(name="u", bufs=BPC))
        st_pool = ctx.enter_context(tc.tile_pool(name="st", bufs=2))
        f1_pool = ctx.enter_context(tc.tile_pool(name="f1", bufs=2))
        f2_pool = ctx.enter_context(tc.tile_pool(name="f2", bufs=4))
        s_pool = ctx.enter_context(tc.tile_pool(name="s", bufs=6))
        scr_pool = ctx.enter_context(tc.tile_pool(name="scr", bufs=1))
        o_pool = ctx.enter_context(tc.tile_pool(name="o", bufs=2))
        psum = ctx.enter_context(tc.tile_pool(name="psum", bufs=4, space="PSUM"))
        psum2 = ctx.enter_context(tc.tile_pool(name="psum2", bufs=2, space="PSUM"))
        psum_s = ctx.enter_context(tc.tile_pool(name="psum_s", bufs=2, space="PSUM"))

        # ---- constants & weights (outside the timing loop) ----
        w1_sb = singles.tile([C, NCHUNK, H], F16)
        nc.sync.dma_start(out=w1_sb[:], in_=w1_ext.rearrange("k p m -> p k m"))
        w2_sb = singles.tile([C, 2, NCHUNK, C], F16)
        nc.sync.dma_start(out=w2_sb[:], in_=w2_ext.rearrange("g k p m -> p g k m"))
        ones_sb = singles.tile([128, 128], F32)
        nc.vector.memset(ones_sb[:], 1.0 / 128.0)
        eps_sb = singles.tile([128, 1], F32)
        nc.vector.memset(eps_sb[:], LN_EPS)
        if ln_affine:
            lnw_sb = singles.tile([C, T], F32)
            nc.sync.dma_start(out=lnw_sb[:], in_=lnw_ext[:])
            lnb_sb = singles.tile([C, T], F32)
            nc.sync.dma_start(out=lnb_sb[:], in_=lnb_ext[:])

        # per-batch LN coefficients: [inv, a_s, b_s, nb]
        coef = singles.tile([128, BPC, 4], F32)

        loop_cm = tc.For_i(0, n_loop, 1) if n_loop > 1 else nullcontext()
        with loop_cm:
            u_tiles = []
            for b in range(BPC):
                u_t = u_pool.tile([C, T], F32, tag="u")
                nc.sync.dma_start(out=u_t[:], in_=u_ext[b])
                u_tiles.append(u_t)

                # ---- LN stats for batch b ----
                st = st_pool.tile([128, 6], F32, tag="bnst")
                nc.vector.bn_stats(out=st[:], in_=u_t[:])
                mv = st_pool.tile([128, 2], F32, tag="bnmv")
                nc.vector.bn_aggr(out=mv[:], in_=st[:])
                m2 = st_pool.tile([128, 1], F32, tag="m2")
                nc.vector.tensor_tensor(out=m2[:], in0=mv[:, 0:1], in1=mv[:, 0:1], op=Op.mult)
                nc.vector.tensor_tensor(out=mv[:, 1:2], in0=mv[:, 1:2], in1=m2[:], op=Op.add)
                ps = psum_s.tile([128, 2], F32, tag="ps_st")
                nc.tensor.matmul(ps[:], lhsT=ones_sb[:], rhs=mv[:], start=True, stop=True)
                stot = st_pool.tile([128, 2], F32, tag="stot")
                nc.vector.tensor_copy(out=stot[:], in_=ps[:])
                var = st_pool.tile([128, 1], F32, tag="var")
                nc.vector.tensor_tensor(out=var[:], in0=stot[:, 0:1], in1=stot[:, 0:1], op=Op.mult)
                nc.vector.tensor_tensor(out=var[:], in0=stot[:, 1:2], in1=var[:], op=Op.subtract)
                sd = st_pool.tile([128, 1], F32, tag="sd")
                nc.scalar.activation(out=sd[:], in_=var[:], func=Act.Sqrt, bias=eps_sb[:])
                inv = coef[:, b, 0:1]
                nc.vector.reciprocal(out=inv, in_=sd[:])
                mi = st_pool.tile([128, 1], F32, tag="mi")
                nc.vector.tensor_tensor(out=mi[:], in0=stot[:, 0:1], in1=inv, op=Op.mult)
                nc.vector.tensor_scalar(out=coef[:, b, 1:2], in0=inv, scalar1=2.5,
                                        scalar2=None, op0=Op.mult)
                nc.vector.tensor_scalar(out=coef[:, b, 2:3], in0=mi[:], scalar1=-2.5,
                                        scalar2=5.5, op0=Op.mult, op1=Op.add)
                nc.vector.tensor_scalar(out=coef[:, b, 3:4], in0=mi[:], scalar1=-1.0,
                                        scalar2=None, op0=Op.mult)

            def emit_features(s_ap, feats):
                """feats[:, 1+j, :] = -a_j^3/6 ; feats[:, 9+j, :] = -(2/3)b_j^3."""
                v = scr_pool.tile([128, 8, T], F16, tag="v")
                for j in range(8):
                    nc.gpsimd.tensor_scalar(out=v[:, j, :], in0=s_ap,
                                            scalar1=float(j + 2), scalar2=None,
                                            op0=Op.subtract)
                # in-place |.| on the packed (128, 8*T) tile
                nc.vector.tensor_scalar(out=v.bitcast(U16)[:], in0=v.bitcast(U16)[:],
                                        scalar1=0x7FFF, scalar2=None, op0=Op.bitwise_and)
                m = scr_pool.tile([128, 8, T], F16, tag="m")
                nc.vector.tensor_scalar(out=m[:], in0=v[:], scalar1=2.0,
                                        scalar2=0.0, op0=Op.subtract, op1=Op.min)
                n = scr_pool.tile([128, 8, T], F16, tag="n")
                nc.gpsimd.tensor_scalar(out=n[:], in0=m[:], scalar1=1.0,
                                        scalar2=0.0, op0=Op.add, op1=Op.min)
                msq = scr_pool.tile([128, 8, T], F16, tag="msq")
                nc.scalar.activation(out=msq[:], in_=m[:], func=Act.Square, scale=SQ_M)
                nsq = scr_pool.tile([128, 8, T], F16, tag="nsq")
                nc.scalar.activation(out=nsq[:], in_=n[:], func=Act.Square, scale=SQ_N)
                nc.vector.tensor_tensor(out=feats[:, 1:9, :], in0=msq[:], in1=m[:], op=Op.mult)
                nc.vector.tensor_tensor(out=feats[:, 9:17, :], in0=nsq[:], in1=n[:], op=Op.mult)

            for b in range(BPC):
                u_t = u_tiles[b]
                inv = coef[:, b, 0:1]
                a_s = coef[:, b, 1:2]
                b_s = coef[:, b, 2:3]
                nb = coef[:, b, 3:4]

                feats1 = f1_pool.tile([128, NCHUNK, T], F16, tag="feats1")
                if not ln_affine:
                    s1 = s_pool.tile([128, T], F16, tag="s1")
                    nc.vector.tensor_scalar(out=s1[:], in0=u_t[:], scalar1=a_s,
                                            scalar2=b_s, op0=Op.mult, op1=Op.add)
                    nc.scalar.activation(out=feats1[:, 0, :], in_=u_t[:], func=ACT_SILU,
                                         bias=nb, scale=inv)
                else:
                    z = s_pool.tile([128, T], F32, tag="z")
                    nc.vector.tensor_scalar(out=z[:], in0=u_t[:], scalar1=inv,
                                            scalar2=nb, op0=Op.mult, op1=Op.add)
                    nc.vector.tensor_tensor(out=z[:], in0=z[:], in1=lnw_sb[:], op=Op.mult)
                    nc.vector.tensor_tensor(out=z[:], in0=z[:], in1=lnb_sb[:], op=Op.add)
                    s1 = s_pool.tile([128, T], F16, tag="s1")
                    nc.vector.tensor_scalar(out=s1[:], in0=z[:], scalar1=2.5,
                                            scalar2=5.5, op0=Op.mult, op1=Op.add)
                    nc.scalar.activation(out=feats1[:, 0, :], in_=z[:], func=ACT_SILU)

                emit_features(s1[:], feats1)

                # ---- layer 1 matmuls ----
                feats2 = []
                for h in range(2):
                    ps1 = psum.tile([128, T], F32, tag="ps1")
                    for k in range(NCHUNK):
                        nc.tensor.matmul(ps1[:], lhsT=w1_sb[:, k, h * 128:(h + 1) * 128],
                                         rhs=feats1[:, k, :],
                                         start=(k == 0), stop=(k == NCHUNK - 1))
                    f2 = f2_pool.tile([128, NCHUNK, T], F16, tag="feats2")
                    nc.scalar.activation(out=f2[:, 0, :], in_=ps1[:], func=ACT_SILU)
                    s2h = s_pool.tile([128, T], F16, tag="s2")
                    nc.scalar.activation(out=s2h[:], in_=ps1[:], func=Act.Copy,
                                         bias=5.5, scale=2.5)
                    emit_features(s2h[:], f2)
                    feats2.append(f2)

                # ---- layer 2 matmuls ----
                ps2 = psum2.tile([128, T], F32, tag="ps2")
                for h in range(2):
                    for k in range(NCHUNK):
                        nc.tensor.matmul(ps2[:], lhsT=w2_sb[:, h, k, :],
                                         rhs=feats2[h][:, k, :],
                                         start=(h == 0 and k == 0),
                                         stop=(h == 1 and k == NCHUNK - 1))

                # ---- residual + store ----
                o_t = o_pool.tile([128, T], F32, tag="o")
                nc.vector.tensor_tensor(out=o_t[:], in0=ps2[:], in1=u_t[:], op=Op.add)
                nc.sync.dma_start(out=out_ext[b], in_=o_t[:])

    nc.compile()
    return nc


def _get_nc(ln_affine: bool, sim_safe: bool = False, trace_sim: bool = False,
            n_loop: int = 1):
    key = ("nc", ln_affine, sim_safe, trace_sim, n_loop)
    if key not in _CACHE:
        _CACHE[key] = _build(ln_affine, sim_safe, trace_sim, n_loop)
    return _CACHE[key]


def _prep_weights(bw1, sw1, ss1, bw2, sw2, ss2):
    sw1s = sw1 * ss1[:, :, None]               # (H, C, 8)
    sw2s = sw2 * ss2[:, :, None]               # (C, H, 8)
    w1 = np.empty((NCHUNK, C, H), np.float16)
    w1[0] = bw1.T
    for j in range(8):
        w1[1 + j] = -sw1s[:, :, j].T
        w1[9 + j] = +sw1s[:, :, j].T
    w2 = np.empty((2, NCHUNK, 128, C), np.float16)
    for h in range(2):
        w2[h, 0] = bw2[:, h * 128:(h + 1) * 128].T
        for j in range(8):
            w2[h, 1 + j] = -sw2s[:, h * 128:(h + 1) * 128, j].T
            w2[h, 9 + j] = +sw2s[:, h * 128:(h + 1) * 128, j].T
    return w1, w2


def make_in_maps(U, ln_w, ln_b, bw1, sw1, ss1, bw2, sw2, ss2):
    U = np.ascontiguousarray(np.asarray(U, dtype=np.float32))
    ln_affine = not (np.all(ln_w == 1.0) and np.all(ln_b == 0.0))
    w1, w2 = _prep_weights(np.asarray(bw1, np.float32), np.asarray(sw1, np.float32),
                           np.asarray(ss1, np.float32), np.asarray(bw2, np.float32),
                           np.asarray(sw2, np.float32), np.asarray(ss2, np.float32))
    shards = U.reshape(N_CORES, BPC, C, T)
    in_maps = []
    for c in range(N_CORES):
        m = {"u": shards[c], "w1": w1, "w2": w2}
        if ln_affine:
            m["lnw"] = np.ascontiguousarray(np.asarray(ln_w, np.float32).T)
            m["lnb"] = np.ascontiguousarray(np.asarray(ln_b, np.float32).T)
        in_maps.append(m)
    return in_maps, ln_affine


def run_in_maps(in_maps, ln_affine, n_loop: int = 1):
    from concourse.bass_utils import run_bass_kernel_spmd
    nc = _get_nc(ln_affine, n_loop=n_loop)
    res = run_bass_kernel_spmd(nc, in_maps, core_ids=list(range(N_CORES)))
    return res


def kernel(U, ln_w, ln_b, bw1, sw1, ss1, bw2, sw2, ss2):
    in_maps, ln_affine = make_in_maps(U, ln_w, ln_b, bw1, sw1, ss1, bw2, sw2, ss2)
    res = run_in_maps(in_maps, ln_affine)
    out = np.concatenate([res.results[c]["out"] for c in range(N_CORES)], axis=0)
    return out.reshape(B, C, T).astype(np.float32)
